# revision 6
# baseline (speedup 1.0000x reference)
"""Causal single-head attention (B=4, T=4096, E=204, H=64) on 8 NeuronCores.

Sharding: data-parallel over batch (2 cores per batch element); each core
handles the interleaved half of the 128-row query tiles of its batch. For
odd-parity cores the host swaps adjacent 128-row tile pairs of x so that the
causal loop structure (extents + masks) is identical across all 8 cores -->
one SPMD program, balanced work.

Per-core pipeline (all fp32):
  x --(PE transpose via identity matmul)--> x^T
  Q^T/K^T/V^T = W^T @ x^T          (PE, contraction over E=128+76 chunks)
  V_aug = [V | 1]                   (PE transpose of V^T + ones column)
  for kt (k-tiles, k-outer):        S^T = K_tile @ Q^T   (PE -> PSUM)
      mask diag/pad (DVE add) ; P = exp(scale*S^T) (ACT, PSUM->SBUF)
      acc += V_aug^T @ P            (PE; row 64 accumulates softmax denom)
  out = acc^T[:, :64] * (1/acc^T[:, 64])  (PE transpose + DVE recip/mul)
"""

import sys

if "/opt/trn_rl_repo" not in sys.path:
    sys.path.insert(0, "/opt/trn_rl_repo")

import numpy as np

NEG = -1.0e30
B, T, E, H = 4, 4096, 204, 64
E1 = 128
E2 = E - E1  # 76
NT = T // 128  # 32 k-tiles
NCORES = 8
SCALE = 1.0 / float(np.sqrt(E))

_CACHE = {}


def _build_nc():
    from contextlib import ExitStack

    import concourse.bacc as bacc
    import concourse.bass as bass
    import concourse.mybir as mybir
    import concourse.tile as tile
    from concourse.masks import make_identity

    f32 = mybir.dt.float32
    Exp = mybir.ActivationFunctionType.Exp

    nc = bacc.Bacc("TRN2", target_bir_lowering=False, debug=False)

    xin = nc.dram_tensor("xin", [T, E], f32, kind="ExternalInput")
    wq_d = nc.dram_tensor("wq", [E, H], f32, kind="ExternalInput")
    wk_d = nc.dram_tensor("wk", [E, H], f32, kind="ExternalInput")
    wv_d = nc.dram_tensor("wv", [E, H], f32, kind="ExternalInput")
    pad_d = nc.dram_tensor("pad", [128, 1], f32, kind="ExternalInput")
    y_d = nc.dram_tensor("y", [T // 2, H], f32, kind="ExternalOutput")

    with tile.TileContext(nc) as tc, ExitStack() as ctx:
        const = ctx.enter_context(tc.tile_pool(name="const", bufs=1))
        big = ctx.enter_context(tc.tile_pool(name="big", bufs=1))
        xstg = ctx.enter_context(tc.tile_pool(name="xstg", bufs=4))
        ppool = ctx.enter_context(tc.tile_pool(name="pp", bufs=3))
        oapool = ctx.enter_context(tc.tile_pool(name="oa", bufs=2))
        ypool = ctx.enter_context(tc.tile_pool(name="yp", bufs=3))
        rpool = ctx.enter_context(tc.tile_pool(name="rp", bufs=3))
        spool = ctx.enter_context(
            tc.tile_pool(name="S", bufs=2, space=bass.MemorySpace.PSUM)
        )
        accpool = ctx.enter_context(
            tc.tile_pool(name="acc", bufs=1, space=bass.MemorySpace.PSUM)
        )

        ident = const.tile([128, 128], f32)
        tri = const.tile([128, 128], f32)
        pad_sb = const.tile([128, 1], f32)
        wsb = {}
        for nm, dram in (("wq", wq_d), ("wk", wk_d), ("wv", wv_d)):
            wa = const.tile([E1, H], f32, tag=nm + "a")
            wb = const.tile([E2, H], f32, tag=nm + "b")
            nc.sync.dma_start(wa[:], dram[0:E1, :])
            nc.sync.dma_start(wb[:], dram[E1:E, :])
            wsb[nm] = (wa, wb)
        nc.sync.dma_start(pad_sb[:], pad_d[:])

        make_identity(nc, ident[:])
        # tri[k, q] = 0 if k <= q else NEG   (strict lower triangle masked)
        nc.gpsimd.memset(tri[:], 0.0)
        nc.gpsimd.affine_select(
            out=tri[:],
            in_=tri[:],
            compare_op=mybir.AluOpType.is_ge,
            fill=NEG,
            base=0,
            pattern=[[1, 128]],  # iota = -k + q ; keep where >= 0
            channel_multiplier=-1,
        )

        xT_a = big.tile([E1, T], f32)
        xT_b = big.tile([E2, T], f32)
        QT = big.tile([H, T], f32)
        KT = big.tile([H, T], f32)
        VT = big.tile([H, T], f32)
        vaug = big.tile([128, NT * (H + 1)], f32)

        # ---- x load + PE transpose into x^T ----
        for t in range(NT):
            xt = xstg.tile([128, E], f32)
            nc.sync.dma_start(xt[:], xin[t * 128 : (t + 1) * 128, :])
            pa = spool.tile([128, 1024], f32, tag="S")
            nc.tensor.matmul(
                pa[0:E1, 0:128], xt[:, 0:E1], ident[:], start=True, stop=True
            )
            nc.tensor.matmul(
                pa[0:E2, 512:640], xt[:, E1:E], ident[:], start=True, stop=True
            )
            nc.vector.tensor_copy(xT_a[:, t * 128 : (t + 1) * 128], pa[0:E1, 0:128])
            nc.vector.tensor_copy(xT_b[:, t * 128 : (t + 1) * 128], pa[0:E2, 512:640])

        # ---- projections: dst^T = W^T @ x^T  (accumulate over E chunks) ----
        for nm, dst in (("wq", QT), ("wk", KT), ("wv", VT)):
            wa, wb = wsb[nm]
            for half in range(2):
                ps = [
                    spool.tile([H, 1024], f32, tag="S", name="psproj0"),
                    spool.tile([H, 1024], f32, tag="S", name="psproj1"),
                ]
                for e, (w_, xT_) in enumerate(((wa, xT_a), (wb, xT_b))):
                    for tc_ in range(4):
                        tch = half * 4 + tc_
                        nc.tensor.matmul(
                            ps[tc_ // 2][:, (tc_ % 2) * 512 : (tc_ % 2 + 1) * 512],
                            w_[:],
                            xT_[:, tch * 512 : (tch + 1) * 512],
                            start=(e == 0),
                            stop=(e == 1),
                        )
                nc.vector.tensor_copy(
                    dst[:, half * 2048 : half * 2048 + 1024], ps[0][:]
                )
                nc.vector.tensor_copy(
                    dst[:, half * 2048 + 1024 : (half + 1) * 2048], ps[1][:]
                )

        # ---- V_aug = [V | 1] via PE transpose of V^T ----
        vaug_r = vaug[:].rearrange("p (k c) -> p k c", c=H + 1)
        nc.vector.memset(vaug_r[:, :, H : H + 1], 1.0)
        for kt in range(NT):
            pv = spool.tile([128, 1024], f32, tag="S")
            nc.tensor.matmul(
                pv[:, 0:H],
                VT[:, kt * 128 : (kt + 1) * 128],
                ident[0:H, 0:H],
                start=True,
                stop=True,
            )
            nc.vector.tensor_copy(
                vaug[:, kt * (H + 1) : kt * (H + 1) + H], pv[:, 0:H]
            )

        # ---- attention: k-outer over 32 k-tiles, 4 query chunks of 512 ----
        acc = [accpool.tile([H + 1, 512], f32, name=f"acc{a}") for a in range(4)]
        QT_r = QT[:].rearrange("p (j t) -> p j t", t=256)  # even 128-tiles at [:, j, 0:128]

        for kt in range(NT):
            a_min = kt // 8
            u = kt - 8 * a_min
            v0 = u // 2  # first eligible 128-block of chunk a_min
            elig = list(range(a_min, 4))
            pairs = [elig[i : i + 2] for i in range(0, len(elig), 2)]
            kslice = KT[:, kt * 128 : (kt + 1) * 128]
            vslice = vaug[:, kt * (H + 1) : (kt + 1) * (H + 1)]
            for pair in pairs:
                S = spool.tile([128, 1024], f32, tag="S")
                for idx, a in enumerate(pair):
                    voff = v0 if a == a_min else 0
                    nc.tensor.matmul(
                        S[:, idx * 512 + voff * 128 : (idx + 1) * 512],
                        kslice,
                        QT_r[:, 4 * a + voff : 4 * a + 4, 0:128],
                        start=True,
                        stop=True,
                    )
                if pair[0] == a_min:
                    blk = S[:, v0 * 128 : v0 * 128 + 128]
                    if u % 2 == 0:
                        nc.vector.tensor_add(blk, blk, tri[:])
                    else:
                        nc.vector.tensor_scalar_add(blk, blk, pad_sb[:])
                lo = v0 * 128 if pair[0] == a_min else 0
                hi = len(pair) * 512
                P = ppool.tile([128, 1024], f32)
                nc.scalar.activation(P[:, lo:hi], S[:, lo:hi], Exp, scale=SCALE)
                for idx, a in enumerate(pair):
                    voff = v0 if a == a_min else 0
                    nc.tensor.matmul(
                        acc[a][:, voff * 128 : 512],
                        vslice,
                        P[:, idx * 512 + voff * 128 : (idx + 1) * 512],
                        start=(kt == 0),
                        stop=(kt == 8 * a + 7),
                        skip_group_check=True,
                    )
            if u == 7:
                # chunk a_min complete: normalize + emit its 4 query tiles
                a = a_min
                oa = oapool.tile([H + 1, 512], f32)
                nc.vector.tensor_copy(oa[:], acc[a][:])
                for j in range(4):
                    pf = spool.tile([128, 1024], f32, tag="S")
                    nc.tensor.matmul(
                        pf[:, 0 : H + 1],
                        oa[:, j * 128 : (j + 1) * 128],
                        ident[0 : H + 1, 0 : H + 1],
                        start=True,
                        stop=True,
                    )
                    r = rpool.tile([128, 1], f32)
                    nc.vector.reciprocal(r[:], pf[:, H : H + 1])
                    yt = ypool.tile([128, H], f32)
                    nc.vector.tensor_scalar_mul(yt[:], pf[:, 0:H], r[:])
                    q = 4 * a + j
                    nc.sync.dma_start(y_d[q * 128 : (q + 1) * 128, :], yt[:])

    nc.compile()
    return nc


def _get_nc():
    if "nc" not in _CACHE:
        _CACHE["nc"] = _build_nc()
    return _CACHE["nc"]


_PAIR_SWAP = np.arange(NT).reshape(-1, 2)[:, ::-1].reshape(-1)  # [1,0,3,2,...]


def _make_in_maps(x, Wq, Wk, Wv):
    x = np.asarray(x, dtype=np.float32)
    Wq = np.ascontiguousarray(np.asarray(Wq, dtype=np.float32))
    Wk = np.ascontiguousarray(np.asarray(Wk, dtype=np.float32))
    Wv = np.ascontiguousarray(np.asarray(Wv, dtype=np.float32))
    assert x.shape == (B, T, E)
    in_maps = []
    for c in range(NCORES):
        b, s = c // 2, c % 2
        xb = x[b]
        if s == 1:
            xb = xb.reshape(NT, 128, E)[_PAIR_SWAP].reshape(T, E)
        in_maps.append(
            {
                "xin": np.ascontiguousarray(xb),
                "wq": Wq,
                "wk": Wk,
                "wv": Wv,
                "pad": np.full((128, 1), NEG if s == 0 else 0.0, np.float32),
            }
        )
    return in_maps


def _gather(results):
    y = np.empty((B, T, H), dtype=np.float32)
    for c in range(NCORES):
        b, s = c // 2, c % 2
        yl = np.asarray(results[c]["y"]).reshape(T // 256, 128, H)
        yv = y[b].reshape(NT, 128, H)
        yv[2 * np.arange(T // 256) + s] = yl
    return y


def kernel(x, Wq, Wk, Wv, mask=True, **_ignored):
    assert bool(mask), "kernel compiled for causal (mask=True)"
    nc = _get_nc()
    from concourse import bass_utils

    in_maps = _make_in_maps(x, Wq, Wk, Wv)
    res = bass_utils.run_bass_kernel_spmd(nc, in_maps, list(range(NCORES)))
    _CACHE["last_result"] = res
    return _gather(res.results)


if __name__ == "__main__":
    # smoke test with random data
    rng = np.random.default_rng(0)
    x = rng.standard_normal((B, T, E), dtype=np.float32)
    s = 1.0 / np.sqrt(E)
    Wq = (rng.standard_normal((E, H)) * s).astype(np.float32)
    Wk = (rng.standard_normal((E, H)) * s).astype(np.float32)
    Wv = (rng.standard_normal((E, H)) * s).astype(np.float32)
    out = kernel(x, Wq, Wk, Wv, True)
    print("out", out.shape, out.dtype, float(np.abs(out).max()))


# revision 15
# speedup vs baseline: 1.5683x; 1.5683x over previous
"""Causal single-head attention (B=4, T=4096, E=204, H=64) on 8 NeuronCores.

Sharding: data-parallel over batch (2 cores per batch element); each core
handles the interleaved half of the 128-row query tiles of its batch. For
odd-parity cores the host swaps adjacent 128-row tile pairs of x so that the
causal loop structure (extents + masks) is identical across all 8 cores -->
one SPMD program, balanced work.

Per-core pipeline (all fp32):
  x --(PE transpose via identity matmul)--> x^T
  Q^T/K^T/V^T = W^T @ x^T          (PE, contraction over E=128+76 chunks)
  V_aug = [V | 1]                   (PE transpose of V^T + ones column)
  for kt (k-tiles, k-outer):        S^T = K_tile @ Q^T   (PE -> PSUM)
      mask diag/pad (DVE add) ; P = exp(scale*S^T) (ACT, PSUM->SBUF)
      acc += V_aug^T @ P            (PE; row 64 accumulates softmax denom)
  out = acc^T[:, :64] * (1/acc^T[:, 64])  (PE transpose + DVE recip/mul)
"""

import sys

if "/opt/trn_rl_repo" not in sys.path:
    sys.path.insert(0, "/opt/trn_rl_repo")

import numpy as np

NEG = -1.0e30
B, T, E, H = 4, 4096, 204, 64
E1 = 128
E2 = E - E1  # 76
NT = T // 128  # 32 k-tiles
NCORES = 8
SCALE = 1.0 / float(np.sqrt(E))

_CACHE = {}


def _build_nc():
    from contextlib import ExitStack

    import concourse.bacc as bacc
    import concourse.bass as bass
    import concourse.mybir as mybir
    import concourse.tile as tile
    from concourse.masks import make_identity

    f32 = mybir.dt.float32
    r32 = mybir.dt.float32r  # full-rate PE path (fp32 runs as 2 half-speed MMs)
    Exp = mybir.ActivationFunctionType.Exp

    def R(ap):
        return ap.bitcast(r32)

    nc = bacc.Bacc("TRN2", target_bir_lowering=False, debug=False)

    xin = nc.dram_tensor("xin", [T, E], f32, kind="ExternalInput")
    wq_d = nc.dram_tensor("wq", [E, H], f32, kind="ExternalInput")
    wk_d = nc.dram_tensor("wk", [E, H], f32, kind="ExternalInput")
    wv_d = nc.dram_tensor("wv", [E, H], f32, kind="ExternalInput")
    pad_d = nc.dram_tensor("pad", [128, 1], f32, kind="ExternalInput")
    y_d = nc.dram_tensor("y", [T // 2, H], f32, kind="ExternalOutput")

    with tile.TileContext(nc) as tc, ExitStack() as ctx:
        const = ctx.enter_context(tc.tile_pool(name="const", bufs=1))
        big = ctx.enter_context(tc.tile_pool(name="big", bufs=1))
        xstg = ctx.enter_context(tc.tile_pool(name="xstg", bufs=4))
        ppool = ctx.enter_context(tc.tile_pool(name="pp", bufs=3))
        oapool = ctx.enter_context(tc.tile_pool(name="oa", bufs=2))
        ypool = ctx.enter_context(tc.tile_pool(name="yp", bufs=3))
        rpool = ctx.enter_context(tc.tile_pool(name="rp", bufs=3))
        spool = ctx.enter_context(
            tc.tile_pool(name="S", bufs=2, space=bass.MemorySpace.PSUM)
        )
        accpool = ctx.enter_context(
            tc.tile_pool(name="acc", bufs=1, space=bass.MemorySpace.PSUM)
        )

        ident = const.tile([128, 128], f32)
        tri = const.tile([128, 128], f32)
        pad_sb = const.tile([128, 1], f32)
        wsb = {}
        for nm, dram in (("wq", wq_d), ("wk", wk_d), ("wv", wv_d)):
            wa_f = const.tile([E1, H], f32, tag=nm + "af")
            wb_f = const.tile([E2, H], f32, tag=nm + "bf")
            nc.sync.dma_start(wa_f[:], dram[0:E1, :])
            nc.sync.dma_start(wb_f[:], dram[E1:E, :])
            wa = const.tile([E1, H], r32, tag=nm + "a")
            wb = const.tile([E2, H], r32, tag=nm + "b")
            nc.vector.tensor_copy(wa[:], wa_f[:])
            nc.vector.tensor_copy(wb[:], wb_f[:])
            wsb[nm] = (wa, wb)
        nc.sync.dma_start(pad_sb[:], pad_d[:])

        make_identity(nc, ident[:])
        # tri[k, q] = 0 if k <= q else NEG   (strict lower triangle masked)
        nc.gpsimd.memset(tri[:], 0.0)
        nc.gpsimd.affine_select(
            out=tri[:],
            in_=tri[:],
            compare_op=mybir.AluOpType.is_ge,
            fill=NEG,
            base=0,
            pattern=[[1, 128]],  # iota = -k + q ; keep where >= 0
            channel_multiplier=-1,
        )

        xT_a = big.tile([E1, T], r32)
        xT_b = big.tile([E2, T], r32)
        QT = big.tile([H, T], r32)
        KT = big.tile([H, T], r32)
        VT = big.tile([H, T], f32)
        vaug = big.tile([128, NT * (H + 1)], r32)

        # ---- x load + PE transpose into x^T ----
        for t in range(NT):
            xt = xstg.tile([128, E], f32)
            nc.sync.dma_start(xt[:], xin[t * 128 : (t + 1) * 128, :])
            pa = spool.tile([128, 1024], f32, tag="S")
            nc.tensor.transpose(pa[0:E1, 0:128], xt[:, 0:E1], ident[:])
            nc.tensor.transpose(pa[0:E2, 512:640], xt[:, E1:E], ident[:])
            nc.vector.tensor_copy(xT_a[:, t * 128 : (t + 1) * 128], pa[0:E1, 0:128])
            nc.vector.tensor_copy(xT_b[:, t * 128 : (t + 1) * 128], pa[0:E2, 512:640])

        # ---- projections: dst^T = W^T @ x^T  (accumulate over E chunks) ----
        for nm, dst in (("wq", QT), ("wk", KT), ("wv", VT)):
            wa, wb = wsb[nm]
            for half in range(2):
                ps = [
                    spool.tile([H, 1024], f32, tag="S", name="psproj0"),
                    spool.tile([H, 1024], f32, tag="S", name="psproj1"),
                ]
                for e, (w_, xT_) in enumerate(((wa, xT_a), (wb, xT_b))):
                    for tc_ in range(4):
                        tch = half * 4 + tc_
                        nc.tensor.matmul(
                            ps[tc_ // 2][:, (tc_ % 2) * 512 : (tc_ % 2 + 1) * 512],
                            w_[:],
                            xT_[:, tch * 512 : (tch + 1) * 512],
                            start=(e == 0),
                            stop=(e == 1),
                        )
                nc.vector.tensor_copy(
                    dst[:, half * 2048 : half * 2048 + 1024], ps[0][:]
                )
                nc.vector.tensor_copy(
                    dst[:, half * 2048 + 1024 : (half + 1) * 2048], ps[1][:]
                )

        # ---- V_aug = [V | 1] via PE transpose of V^T ----
        vaug_r = vaug[:].rearrange("p (k c) -> p k c", c=H + 1)
        ones = const.tile([128, NT], f32)
        nc.vector.memset(ones[:], 1.0)
        nc.vector.tensor_copy(
            vaug_r[:, :, H : H + 1],
            ones[:].rearrange("p (k o) -> p k o", o=1),
        )
        for kt in range(NT):
            pv = spool.tile([128, 1024], f32, tag="S")
            nc.tensor.transpose(
                pv[:, 0:H], VT[:, kt * 128 : (kt + 1) * 128], ident[0:H, 0:H]
            )
            nc.vector.tensor_copy(
                vaug[:, kt * (H + 1) : kt * (H + 1) + H], pv[:, 0:H]
            )

        # ---- attention: k-outer over 32 k-tiles, 4 query chunks of 512 ----
        acc = [accpool.tile([H + 1, 512], f32, name=f"acc{a}") for a in range(4)]
        QT_r = QT[:].rearrange("p (j t) -> p j t", t=256)  # even 128-tiles at [:, j, 0:128]

        for kt in range(NT):
            a_min = kt // 8
            u = kt - 8 * a_min
            v0 = u // 2  # first eligible 128-block of chunk a_min
            elig = list(range(a_min, 4))
            pairs = [elig[i : i + 2] for i in range(0, len(elig), 2)]
            kslice = KT[:, kt * 128 : (kt + 1) * 128]
            vslice = vaug[:, kt * (H + 1) : (kt + 1) * (H + 1)]
            for pair in pairs:
                S = spool.tile([128, 1024], f32, tag="S")
                for idx, a in enumerate(pair):
                    voff = v0 if a == a_min else 0
                    nc.tensor.matmul(
                        S[:, idx * 512 + voff * 128 : (idx + 1) * 512],
                        kslice,
                        QT_r[:, 4 * a + voff : 4 * a + 4, 0:128],
                        start=True,
                        stop=True,
                    )
                if pair[0] == a_min:
                    blk = S[:, v0 * 128 : v0 * 128 + 128]
                    if u % 2 == 0:
                        nc.vector.tensor_add(blk, blk, tri[:])
                    else:
                        nc.vector.tensor_scalar_add(blk, blk, pad_sb[:])
                lo = v0 * 128 if pair[0] == a_min else 0
                hi = len(pair) * 512
                P = ppool.tile([128, 1024], r32)
                nc.scalar.activation(P[:, lo:hi], S[:, lo:hi], Exp, scale=SCALE)
                for idx, a in enumerate(pair):
                    voff = v0 if a == a_min else 0
                    nc.tensor.matmul(
                        acc[a][:, voff * 128 : 512],
                        vslice,
                        P[:, idx * 512 + voff * 128 : (idx + 1) * 512],
                        start=(kt == 0),
                        stop=(kt == 8 * a + 7),
                        skip_group_check=True,
                    )
            if u == 7:
                # chunk a_min complete: normalize + emit its 4 query tiles
                a = a_min
                oa = oapool.tile([H + 1, 512], f32)
                nc.vector.tensor_copy(oa[:], acc[a][:])
                for j in range(4):
                    pf = spool.tile([128, 1024], f32, tag="S")
                    nc.tensor.transpose(
                        pf[:, 0 : H + 1],
                        oa[:, j * 128 : (j + 1) * 128],
                        ident[0 : H + 1, 0 : H + 1],
                    )
                    r = rpool.tile([128, 1], f32)
                    nc.vector.reciprocal(r[:], pf[:, H : H + 1])
                    yt = ypool.tile([128, H], f32)
                    nc.vector.tensor_scalar_mul(yt[:], pf[:, 0:H], r[:])
                    q = 4 * a + j
                    nc.sync.dma_start(y_d[q * 128 : (q + 1) * 128, :], yt[:])

    nc.compile()
    return nc


def _get_nc():
    if "nc" not in _CACHE:
        _CACHE["nc"] = _build_nc()
    return _CACHE["nc"]


_PAIR_SWAP = np.arange(NT).reshape(-1, 2)[:, ::-1].reshape(-1)  # [1,0,3,2,...]


def _make_in_maps(x, Wq, Wk, Wv):
    x = np.asarray(x, dtype=np.float32)
    Wq = np.ascontiguousarray(np.asarray(Wq, dtype=np.float32))
    Wk = np.ascontiguousarray(np.asarray(Wk, dtype=np.float32))
    Wv = np.ascontiguousarray(np.asarray(Wv, dtype=np.float32))
    assert x.shape == (B, T, E)
    in_maps = []
    for c in range(NCORES):
        b, s = c // 2, c % 2
        xb = x[b]
        if s == 1:
            xb = xb.reshape(NT, 128, E)[_PAIR_SWAP].reshape(T, E)
        in_maps.append(
            {
                "xin": np.ascontiguousarray(xb),
                "wq": Wq,
                "wk": Wk,
                "wv": Wv,
                "pad": np.full((128, 1), NEG if s == 0 else 0.0, np.float32),
            }
        )
    return in_maps


def _gather(results):
    y = np.empty((B, T, H), dtype=np.float32)
    for c in range(NCORES):
        b, s = c // 2, c % 2
        yl = np.asarray(results[c]["y"]).reshape(T // 256, 128, H)
        yv = y[b].reshape(NT, 128, H)
        yv[2 * np.arange(T // 256) + s] = yl
    return y


def kernel(x, Wq, Wk, Wv, mask=True, **_ignored):
    assert bool(mask), "kernel compiled for causal (mask=True)"
    nc = _get_nc()
    from concourse import bass_utils

    in_maps = _make_in_maps(x, Wq, Wk, Wv)
    res = bass_utils.run_bass_kernel_spmd(nc, in_maps, list(range(NCORES)))
    _CACHE["last_result"] = res
    return _gather(res.results)


if __name__ == "__main__":
    # smoke test with random data
    rng = np.random.default_rng(0)
    x = rng.standard_normal((B, T, E), dtype=np.float32)
    s = 1.0 / np.sqrt(E)
    Wq = (rng.standard_normal((E, H)) * s).astype(np.float32)
    Wk = (rng.standard_normal((E, H)) * s).astype(np.float32)
    Wv = (rng.standard_normal((E, H)) * s).astype(np.float32)
    out = kernel(x, Wq, Wk, Wv, True)
    print("out", out.shape, out.dtype, float(np.abs(out).max()))
